# revision 56
# baseline (speedup 1.0000x reference)
"""DMSAD loss kernel for Trainium2 (8 NeuronCores, data-parallel over batch).

Computes mean over B rows of:
    dist_i = max(min_j ||x_i - c_j||^2, 0)
    loss_i = dist_i                 if st_i == 0
             dist_i + EPS           if st_i == 1
             1 / (dist_i + EPS)     if st_i == -1

Per core (B_SH = 16384 rows, D = 256, C = 128), engine pipeline:
  - DMA (SWDGE/gpsimd): casting fp32->bf16 HBM loads -- the 16.8 MB fp32
    read paces at the ~47us HBM roofline while landing bf16 directly in
    SBUF, deleting the old ACT/DVE cast stage entirely.  st is permuted
    on the HOST to the x row mapping so it loads with one contiguous DMA
    (the scattered layout was 2048 32-byte descriptors / ~14us).
  - PE: a 12-matmul warmup burst holds the HAM clock gate at 2.4 GHz
    before the stream starts (the gate defaults to 1.2 GHz and needs
    ~3.4us of sustained matmul activity); per group: transpose-mode
    128x128 transposes (bf16 stays bf16 in PSUM), then per 4-tile bank a
    K=1 ones x c2_bf16 matmul + main bf16 matmuls G += -2 x.c^T + a
    rank-1 ones-matmul of the squared transposed tiles that folds x2
    into G, so dist = min_c(G) directly.  The matmul phase runs one
    group BEHIND the transpose/copy/square phase (software pipeline,
    per-chunk 1-bank G tiles with 4-deep rotation) so the PE streams a
    group's matmuls while the next group's copies are in flight.
  - ACT: PSUM->SBUF copies of transposed x as bitcast fp32 pairs.
  - DVE: sqt squares (bf16 2x), per-chunk min-reduce over centers, and
    the endgame in 5 chunks overlapped with the main loop (st-only
    terms precomputed; eps-part folded into one reduce).
  - The first RAMP_SQRED groups compute x2 on the idle DVE instead of
    the PE (square+fold+reduce) -- during the DMA ramp the PE is
    data-starved, and trimming its work below the arrival rate removes
    the idle gaps that used to re-throttle the HAM clock mid-ramp; a few
    filler matmuls in the ramp->steady transition pocket bridge the last
    such gap, keeping the clock at 2.4 GHz through the whole stream.
A final ones-matmul collapses the per-partition loss sums to one scalar
per core (single-descriptor 4-byte out DMA); host adds the 8 partials.
"""

from contextlib import ExitStack, nullcontext

import numpy as np

import concourse.bass as bass
import concourse.tile as tile
from concourse import bacc, mybir
from concourse.bass_utils import run_bass_kernel_spmd
from concourse.masks import make_identity

N_CORES = 8
B = 131072
D = 256
C = 128
P = 128
B_SH = B // N_CORES          # 16384 rows per core
NT = B_SH // P               # 128 b-tiles of 128 rows
MINI = 2                     # b-tiles per transpose PSUM minigroup (1 bank fp32)
PSUM_GROUP = 4               # b-tiles per G PSUM bank
DMA_GROUP = 8                # b-tiles per input DMA (1 MiB fp32 reads)
G_TILES = 8                  # b-tiles per G PSUM tile (2 banks, one min-reduce)
ETA = 1.0
EPS = 1e-6

# ---- engine-balance knobs -------------------------------------------------
# dummy N=512 matmuls at kernel start: the HAM clock gate defaults to the
# cold 1.2 GHz state and only warms after ~3.4us of sustained matmul
# activity; without these the whole ramp phase runs at half clock
WARMUP_MMS = 12
# group layout regions (start tile, end tile, tiles per group): small
# groups at the head keep PE work arriving densely through the DMA ramp
# (gaps >~1us there re-throttle the HAM clock gate), small groups at the
# tail drain the pipeline faster.  _st_index() derives the host-side st
# permutation from the same table.
REGIONS = ((0, 16, 4), (16, 112, 8), (112, 128, 4))
# first RAMP_SQRED groups compute x2 on DVE (sqred) instead of the PE:
# during the DMA ramp the PE is data-starved, and trimming its per-group
# work to the ramp arrival rate avoids the idle gaps that re-throttle the
# HAM clock gate (DVE is idle during the ramp)
RAMP_SQRED = 3
# endgame trigger points (t0+ntile values) and column chunks
EG_PLAN = [(32, 0, 32), (64, 32, 64), (96, 64, 96), (120, 96, 120)]
EG_FINAL = (120, 128)

F32 = mybir.dt.float32
BF16 = mybir.dt.bfloat16
AF = mybir.ActivationFunctionType
ALU = mybir.AluOpType

_cached_nc = {}


def _emit(ctx: ExitStack, tc, x_d, c_d, st_d, out_d, repeat: int = 1,
          hw_loop: int = 1):
    nc = tc.nc

    const = ctx.enter_context(tc.tile_pool(name="const", bufs=1))
    xbpool = ctx.enter_context(tc.tile_pool(name="xb", bufs=8))
    sqpool = ctx.enter_context(tc.tile_pool(name="sq", bufs=4))
    xtps = ctx.enter_context(tc.tile_pool(name="xtps", bufs=3, space="PSUM"))
    xtsb = ctx.enter_context(tc.tile_pool(name="xtsb", bufs=4))
    # G lives in 1-bank [P, 4, C] tiles (per-chunk min-reduce): four of
    # them rotate, giving the one-group-behind matmul pipeline ~3 chunks
    # of bank-reuse slack so the PE never waits on a min-reduce.
    gps = ctx.enter_context(tc.tile_pool(name="gps", bufs=4, space="PSUM"))
    scr_ps = ctx.enter_context(tc.tile_pool(name="scrps", bufs=1, space="PSUM"))
    endp = ctx.enter_context(tc.tile_pool(name="endp", bufs=1))

    # ---- one-time prep -------------------------------------------------
    # HAM warmup: dense matmuls on scratch data so the PE clock is at
    # 2.4 GHz by the time the real stream starts (~10us in)
    warm_rhs = const.tile([P, 4, P], BF16)
    nc.vector.memset(warm_rhs[:], 1.0)
    warm_ps = scr_ps.tile([P, 4, P], F32, tag="scratch")
    for _ in range(WARMUP_MMS):
        nc.tensor.matmul(warm_ps[:].rearrange("p t c -> p (t c)"),
                         lhsT=warm_rhs[:, 0, :],
                         rhs=warm_rhs[:].rearrange("p t c -> p (t c)"),
                         start=True, stop=True)

    # x loads are casting fp32->bf16 SWDGE DMAs (gpsimd): queue the first
    # groups immediately -- they need no prep and pace the whole kernel.
    pre_x8 = []
    for gd in range(3):
        lo, hi, tt = REGIONS[0]
        t0 = lo + gd * tt
        src0 = x_d[t0 * P:(t0 + tt) * P, :]
        src0 = src0.rearrange("(p t) d -> p t d", t=tt)
        x80 = xbpool.tile([P, DMA_GROUP, D], BF16, tag="xb")
        nc.gpsimd.dma_start(x80[:, :tt, :], src0)
        pre_x8.append(x80)

    ident_bf = const.tile([P, P], BF16)
    make_identity(nc, ident_bf[:])
    ident_f32 = const.tile([P, P], F32)
    make_identity(nc, ident_f32[:])

    # warm the ACT Square table set while DMAs are in flight (the
    # ACT_TABLE_LOAD costs ~2.7us and would otherwise sit in the prep
    # critical path at first use)
    warm = const.tile([1, 1], F32)
    nc.scalar.activation(warm[:], ident_f32[0:1, 0:1], AF.Square)

    c_sb = const.tile([C, D], F32)
    nc.sync.dma_start(c_sb[:], c_d[:])

    # c2 = rowsum(c^2) as a [128, 1] fp32 column
    c_sq = const.tile([C, D], F32)
    c2col = const.tile([C, 1], F32)
    nc.scalar.activation(c_sq[:], c_sb[:], AF.Square, accum_out=c2col[:])

    # (-2c) in bf16, then its transpose cT [d-chunk partitions, k, centers]
    cm2 = const.tile([C, D], BF16)
    nc.vector.tensor_scalar_mul(cm2[:], c_sb[:], -2.0)
    ct_ps = scr_ps.tile([P, 2, C], BF16, tag="scratch")
    for k in range(2):
        nc.tensor.transpose(ct_ps[:, k, :], cm2[:, k * P:(k + 1) * P], ident_bf[:])
    cT = const.tile([P, 2, C], BF16)
    nc.vector.tensor_copy(cT[:], ct_ps[:])

    # c2 as a single bf16 K-row for a K=1 ones-matmul (the old fp32-exact
    # hi+lo split needed a gpsimd casting DMA whose queue position stalled
    # the x-load triggers behind the whole prep chain; bf16 c2 costs ~1e-3
    # of the 2e-2 error budget)
    c2t_ps = scr_ps.tile([1, C], F32, tag="scratch")
    nc.tensor.transpose(c2t_ps[:], c2col[:], ident_f32[:])
    c2row_f = const.tile([1, C], F32)
    nc.vector.tensor_copy(c2row_f[:], c2t_ps[:])

    ones2 = const.tile([1, C], BF16)
    nc.vector.memset(ones2[:], 1.0)
    ones_col = const.tile([P, 1], F32)
    nc.vector.memset(ones_col[:], 1.0)

    # c2 row replicated PSUM_GROUP times for the single N=512 c2 matmul
    c2rows4 = const.tile([1, PSUM_GROUP, C], BF16)
    nc.vector.tensor_copy(c2rows4[:, 0, :], c2row_f[:])
    nc.vector.tensor_copy(c2rows4[:, 1, :], c2rows4[:, 0, :])
    nc.vector.tensor_copy(c2rows4[:, 2:4, :], c2rows4[:, 0:2, :])

    # all-ones [d, c] rhs for the PE-side x2 rank-1 accumulation
    ones_dc = const.tile([P, C], BF16)
    nc.vector.memset(ones_dc[:], 1.0)

    # semi_target: the HOST pre-permutes st into the x row mapping
    # (st_pre[p*NT + col] = st[row(p, col)], see make_in_maps), so one
    # contiguous 512B-per-partition DMA loads it.  The old direct load
    # of the scattered layout was 2048 32-byte descriptors (~14us) and
    # stalled the DVE queue behind the endgame's st-dependent ops.
    st_sb = const.tile([P, NT], F32)
    nc.sync.dma_start(st_sb[:], st_d[:].rearrange("(p j) -> p j", p=P))

    # per-b-tile accumulators: column j <-> b-tile j, partition p <-> row in tile
    mw = const.tile([P, NT], F32)
    x2w = const.tile([P, NT], F32)
    nc.vector.memset(x2w[:], 0.0)
    n_eg = len(EG_PLAN) + 1
    lsum2 = const.tile([P, n_eg], F32)

    # x2 rides in G via the PE for every group, so dist = max(mw, 0), and
    # loss = dist + min(st,0)*(dist - r) + max(st,0)*EPS  (r = 1/(dist+EPS))
    #      = dist*(1 + mneg) - mneg*r + epsq
    # with the st-only terms precomputed once here (mneg, m1 = 1+mneg) and
    # the epsq part folded into a single column sum (seg).
    m1 = const.tile([P, NT], F32)
    nc.vector.tensor_scalar(m1[:], st_sb[:], 0.0, 1.0, op0=ALU.min,
                            op1=ALU.add)
    mneg = const.tile([P, NT], F32)
    nc.vector.tensor_scalar_min(mneg[:], st_sb[:], 0.0)
    epsq = endp.tile([P, NT], F32, tag="epsq")
    nc.vector.tensor_scalar(epsq[:], st_sb[:], 0.0, EPS, op0=ALU.max,
                            op1=ALU.mult)
    seg = const.tile([P, 1], F32)
    nc.vector.tensor_reduce(seg[:], epsq[:], axis=mybir.AxisListType.X,
                            op=ALU.add)

    # ---- endgame (runs in chunks; all but the last overlap the main loop)
    def endgame_chunk(h, lo, hi):
        cols = slice(lo, hi)
        W = hi - lo
        dist = endp.tile([P, W], F32, tag=f"dist{h}")
        if lo < RAMP_SQRED * 4:
            # ramp groups keep x2 out of G (sqred path)
            nc.vector.tensor_tensor(dist[:], x2w[:, cols], mw[:, cols],
                                    op=ALU.add)
            nc.vector.tensor_scalar_max(dist[:], dist[:], 0.0)
        else:
            nc.vector.tensor_scalar_max(dist[:], mw[:, cols], 0.0)
        dp = endp.tile([P, W], F32, tag=f"dp{h}")
        nc.vector.tensor_scalar_add(dp[:], dist[:], EPS)
        r = endp.tile([P, W], F32, tag=f"r{h}")
        nc.vector.reciprocal(r[:], dp[:])
        a = endp.tile([P, W], F32, tag=f"a{h}")
        nc.vector.tensor_tensor(a[:], dist[:], m1[:, cols], op=ALU.mult)
        b = endp.tile([P, W], F32, tag=f"b{h}")
        nc.vector.tensor_tensor(b[:], r[:], mneg[:, cols], op=ALU.mult)
        losses = endp.tile([P, W], F32, tag=f"lo{h}")
        nc.vector.tensor_tensor(losses[:], a[:], b[:], op=ALU.subtract)
        nc.vector.tensor_reduce(lsum2[:, h:h + 1], losses[:],
                                axis=mybir.AxisListType.X, op=ALU.add)

    # ---- main loop -----------------------------------------------------
    if repeat == 1 and hw_loop == 1:
        plan = [(t, tt) for lo, hi, tt in REGIONS for t in range(lo, hi, tt)]
    else:
        plan = [(t, 8) for t in range(0, NT, 8)]

    def front_phase(pi, t0, ntile):
        """DMA + transposes + copies + squares for one group."""
        src = x_d[t0 * P:(t0 + ntile) * P, :]
        # row (p, t) of this group = batch t0*128 + p*ntile + t: each
        # partition reads one contiguous run per DMA
        src = src.rearrange("(p t) d -> p t d", t=ntile)
        if repeat == 1 and hw_loop == 1 and pi < len(pre_x8):
            x8 = pre_x8[pi]
        else:
            x8 = xbpool.tile([P, DMA_GROUP, D], BF16, tag="xb")
            nc.gpsimd.dma_start(x8[:, :ntile, :], src)

        cols = slice(t0, t0 + ntile)
        x2_on_pe = pi >= RAMP_SQRED
        chunks = [(s, min(s + PSUM_GROUP, ntile)) for s in
                  range(0, ntile, PSUM_GROUP)]

        if not x2_on_pe:
            # sqred path on DVE: TT square (bf16 2x), half-fold, reduce
            sq = sqpool.tile([P, DMA_GROUP, D], BF16, tag="sq")
            nc.vector.tensor_tensor(sq[:, :ntile, :], x8[:, :ntile, :],
                                    x8[:, :ntile, :], op=ALU.mult)
            s1 = sqpool.tile([P, DMA_GROUP, P], BF16, tag="s1")
            nc.vector.tensor_tensor(
                s1[:, :ntile, :], sq[:, :ntile, 0:P], sq[:, :ntile, P:D],
                op=ALU.add,
            )
            nc.vector.tensor_reduce(
                x2w[:, cols], s1[:, :ntile, :], axis=mybir.AxisListType.X,
                op=ALU.add,
            )

        # all transposes of the group (PE streams them densely)
        xt_pss = []
        for s, e in chunks:
            w = e - s
            xt_ps = xtps.tile([P, PSUM_GROUP, 2, P], BF16)
            for i in range(w):
                for k in range(2):
                    nc.tensor.transpose(
                        xt_ps[:, i, k, :], x8[:, s + i, k * P:(k + 1) * P],
                        ident_bf[:],
                    )
            xt_pss.append(xt_ps)
        # PSUM->SBUF pair copies (ACT)
        xt_ts = []
        for ci, (s, e) in enumerate(chunks):
            w = e - s
            xt_t = xtsb.tile([P, PSUM_GROUP, 2, P], BF16)
            nc.scalar.copy(xt_t[:, :w].bitcast(F32),
                           xt_pss[ci][:, :w].bitcast(F32))
            xt_ts.append(xt_t)
        return [t0, ntile, cols, x2_on_pe, chunks, xt_ts, None]

    def sqt_phase(state):
        """squares of the transposed tiles (DVE, bf16 2x) -- emitted
        AFTER the previous group's back_phase so the min-reduces sit in
        front of them in the DVE FIFO; otherwise the next group's
        x2-matmuls wait ~320ns on a queue-delayed sqt."""
        t0, ntile, cols, x2_on_pe, chunks, xt_ts, _ = state
        sqts = []
        for ci, (s, e) in enumerate(chunks):
            if not x2_on_pe:
                sqts.append(None)
                continue
            w = e - s
            sqt = sqpool.tile([P, PSUM_GROUP, 2, P], BF16, tag="sqt")
            nc.vector.tensor_tensor(sqt[:, :w], xt_ts[ci][:, :w],
                                    xt_ts[ci][:, :w], op=ALU.mult)
            sqts.append(sqt)
        state[6] = sqts

    def back_phase(state):
        """G matmuls + per-chunk min-reduce + endgame triggers, emitted
        one group behind front_phase: the PE streams this group's
        matmuls while the NEXT group's copies are in flight, removing
        the ~320ns copy-latency stall per bank."""
        t0, ntile, cols, x2_on_pe, chunks, xt_ts, sqts = state
        for ci, (s, e) in enumerate(chunks):
            w = e - s
            g_ch = gps.tile([P, PSUM_GROUP, C], F32)
            nc.tensor.matmul(
                g_ch[:, :w].rearrange("p t c -> p (t c)"),
                lhsT=ones2[:],
                rhs=c2rows4[:, :w].rearrange("p t c -> p (t c)"),
                start=True, stop=False,
            )
            for i in range(w):
                last_tile = i == w - 1
                nc.tensor.matmul(
                    g_ch[:, i, :], lhsT=xt_ts[ci][:, i, 0, :],
                    rhs=cT[:, 0, :], start=False, stop=False,
                )
                nc.tensor.matmul(
                    g_ch[:, i, :], lhsT=xt_ts[ci][:, i, 1, :],
                    rhs=cT[:, 1, :], start=False,
                    stop=(last_tile and not x2_on_pe),
                )
            if x2_on_pe:
                for i in range(w):
                    nc.tensor.matmul(
                        g_ch[:, i, :], lhsT=sqts[ci][:, i, 0, :],
                        rhs=ones_dc[:], start=False, stop=False,
                    )
                    nc.tensor.matmul(
                        g_ch[:, i, :], lhsT=sqts[ci][:, i, 1, :],
                        rhs=ones_dc[:], start=False, stop=(i == w - 1),
                    )
            nc.vector.tensor_reduce(
                mw[:, t0 + s:t0 + e], g_ch[:, :w, :],
                axis=mybir.AxisListType.X, op=ALU.min,
            )

        if repeat == 1 and hw_loop == 1:
            for h, (trig, lo, hi) in enumerate(EG_PLAN):
                if t0 + ntile == trig:
                    endgame_chunk(h, lo, hi)

    with tc.For_i(0, hw_loop, 1) if hw_loop > 1 else nullcontext():
     for _rep in range(repeat):
      prev_state = None
      for pi, (t0, ntile) in enumerate(plan):
        state = front_phase(pi, t0, ntile)
        if prev_state is not None:
            back_phase(prev_state)
        if repeat == 1 and hw_loop == 1 and RAMP_SQRED <= pi < RAMP_SQRED + 6:
            # the ramp->steady transition leaves the PE idle for ~2-3us
            # (pipeline startup bubble + DMA still ramping), long enough
            # for the HAM clock gate to re-throttle right before the
            # dense phase; these fillers run in that idle pocket
            for _ in range(3):
                nc.tensor.matmul(warm_ps[:].rearrange("p t c -> p (t c)"),
                                 lhsT=warm_rhs[:, 0, :],
                                 rhs=warm_rhs[:].rearrange("p t c -> p (t c)"),
                                 start=True, stop=True)
        if pi < RAMP_SQRED:
            # no pipelining during the DMA ramp: the PE is data-starved
            # there, and delaying the matmul phase just adds idle gaps
            # that re-throttle the HAM clock gate (ramp groups are
            # sqred, so they need no sqt)
            back_phase(state)
            prev_state = None
        else:
            sqt_phase(state)
            prev_state = state
      back_phase(prev_state)

    endgame_chunk(len(EG_PLAN), *EG_FINAL)
    lacc = lsum2[:, 0:1]
    lsum_t = None
    for h in range(1, n_eg):
        nxt = endp.tile([P, 1], F32, tag=f"ls{h}")
        nc.vector.tensor_tensor(nxt[:], lacc, lsum2[:, h:h + 1], op=ALU.add)
        lacc = nxt[:]
        lsum_t = nxt
    # single-descriptor 4-byte out DMA: a [128,1] out would be 128 tiny
    # descriptors whose completion receipt stalls the end barrier ~7us
    total_ps = scr_ps.tile([1, 1], F32, tag="scratch")
    nc.tensor.matmul(total_ps[:], lhsT=ones_col[:], rhs=lsum_t[:])
    total_sb = endp.tile([1, 1], F32)
    nc.vector.tensor_copy(total_sb[:], total_ps[:])
    nc.sync.dma_start(out_d[:], total_sb[:])


def build_nc(repeat: int = 1, hw_loop: int = 1, internal_x: bool = False):
    key = (repeat, hw_loop, internal_x)
    if key in _cached_nc:
        return _cached_nc[key]
    nc = bacc.Bacc(
        "TRN2",
        target_bir_lowering=False,
        debug=False,
        enable_asserts=False,
        num_devices=N_CORES,
    )
    if internal_x:
        # timing-only builds: x is internal (uninitialized) DRAM so bench
        # calls don't upload 128 MiB; compute timing is data-independent
        x_d = nc.dram_tensor("x", [B_SH, D], F32).ap()
    else:
        x_d = nc.dram_tensor("x", [B_SH, D], F32, kind="ExternalInput").ap()
    c_d = nc.dram_tensor("c", [C, D], F32, kind="ExternalInput").ap()
    st_d = nc.dram_tensor("st", [B_SH], F32, kind="ExternalInput").ap()
    out_d = nc.dram_tensor("out", [1, 1], F32, kind="ExternalOutput").ap()

    with tile.TileContext(nc) as tc:
        with ExitStack() as ctx:
            _emit(ctx, tc, x_d, c_d, st_d, out_d, repeat=repeat, hw_loop=hw_loop)
    nc.compile()
    _cached_nc[key] = nc
    return nc


_ST_IDX = None


def _st_index():
    # row index feeding st_sb[p, col]: in an ntile-tile group at tile t0,
    # batch row t0*128 + p*ntile + t sits at column t0 + t
    global _ST_IDX
    if _ST_IDX is None:
        idx = np.empty((P, NT), dtype=np.int64)
        p = np.arange(P)[:, None]
        for lo, hi, tt in REGIONS:
            for g0 in range(lo, hi, tt):
                t = np.arange(tt)[None, :]
                idx[:, g0:g0 + tt] = g0 * P + p * tt + t
        _ST_IDX = idx.ravel()
    return _ST_IDX


def make_in_maps(x, c, stf):
    idx = _st_index()
    return [
        {
            "x": np.ascontiguousarray(x[i * B_SH:(i + 1) * B_SH]),
            "c": c,
            "st": np.ascontiguousarray(stf[i * B_SH:(i + 1) * B_SH][idx]),
        }
        for i in range(N_CORES)
    ]


def kernel(**inputs) -> np.ndarray:
    x = np.ascontiguousarray(np.asarray(inputs["input"], dtype=np.float32))
    c = np.ascontiguousarray(np.asarray(inputs["c"], dtype=np.float32))
    stf = np.asarray(inputs["semi_target"]).astype(np.float32)

    nc = build_nc()
    res = run_bass_kernel_spmd(nc, make_in_maps(x, c, stf), list(range(N_CORES)))
    total = sum(float(r["out"][0, 0]) for r in res.results)
    return np.asarray(np.float32(total / B))


# revision 57
# speedup vs baseline: 1.0291x; 1.0291x over previous
"""DMSAD loss kernel for Trainium2 (8 NeuronCores, data-parallel over batch).

Computes mean over B rows of:
    dist_i = max(min_j ||x_i - c_j||^2, 0)
    loss_i = dist_i                 if st_i == 0
             dist_i + EPS           if st_i == 1
             1 / (dist_i + EPS)     if st_i == -1

Per core (B_SH = 16384 rows, D = 256, C = 128), engine pipeline:
  - DMA (SWDGE/gpsimd): casting fp32->bf16 HBM loads -- the 16.8 MB fp32
    read paces at the ~47us HBM roofline while landing bf16 directly in
    SBUF, deleting the old ACT/DVE cast stage entirely.  st is permuted
    on the HOST to the x row mapping so it loads with one contiguous DMA
    (the scattered layout was 2048 32-byte descriptors / ~14us).
  - PE: a 12-matmul warmup burst holds the HAM clock gate at 2.4 GHz
    before the stream starts (the gate defaults to 1.2 GHz and needs
    ~3.4us of sustained matmul activity); per group: transpose-mode
    128x128 transposes (bf16 stays bf16 in PSUM), then per 4-tile bank a
    K=1 ones x c2_bf16 matmul + main bf16 matmuls G += -2 x.c^T + a
    rank-1 ones-matmul of the squared transposed tiles that folds x2
    into G, so dist = min_c(G) directly.  The matmul phase runs one
    group BEHIND the transpose/copy/square phase (software pipeline,
    per-chunk 1-bank G tiles with 4-deep rotation) so the PE streams a
    group's matmuls while the next group's copies are in flight.
  - ACT: PSUM->SBUF copies of transposed x as bitcast fp32 pairs.
  - DVE: sqt squares (bf16 2x), per-chunk min-reduce over centers, and
    the endgame in 5 chunks overlapped with the main loop (st-only
    terms precomputed; eps-part folded into one reduce).
  - The first RAMP_SQRED groups compute x2 on the idle DVE instead of
    the PE (square+fold+reduce) -- during the DMA ramp the PE is
    data-starved, and trimming its work below the arrival rate removes
    the idle gaps that used to re-throttle the HAM clock mid-ramp; a few
    filler matmuls in the ramp->steady transition pocket bridge the last
    such gap, keeping the clock at 2.4 GHz through the whole stream.
A final ones-matmul collapses the per-partition loss sums to one scalar
per core (single-descriptor 4-byte out DMA); host adds the 8 partials.
"""

from contextlib import ExitStack, nullcontext

import numpy as np

import concourse.bass as bass
import concourse.tile as tile
from concourse import bacc, mybir
from concourse.bass_utils import run_bass_kernel_spmd
from concourse.masks import make_identity

N_CORES = 8
B = 131072
D = 256
C = 128
P = 128
B_SH = B // N_CORES          # 16384 rows per core
NT = B_SH // P               # 128 b-tiles of 128 rows
MINI = 2                     # b-tiles per transpose PSUM minigroup (1 bank fp32)
PSUM_GROUP = 4               # b-tiles per G PSUM bank
DMA_GROUP = 8                # b-tiles per input DMA (1 MiB fp32 reads)
G_TILES = 8                  # b-tiles per G PSUM tile (2 banks, one min-reduce)
ETA = 1.0
EPS = 1e-6

# ---- engine-balance knobs -------------------------------------------------
# dummy N=512 matmuls at kernel start: the HAM clock gate defaults to the
# cold 1.2 GHz state and only warms after ~3.4us of sustained matmul
# activity; without these the whole ramp phase runs at half clock
WARMUP_MMS = 12
# group layout regions (start tile, end tile, tiles per group): small
# groups at the head keep PE work arriving densely through the DMA ramp
# (gaps >~1us there re-throttle the HAM clock gate), small groups at the
# tail drain the pipeline faster.  _st_index() derives the host-side st
# permutation from the same table.
REGIONS = ((0, 16, 4), (16, 112, 8), (112, 128, 4))
# first RAMP_SQRED groups compute x2 on DVE (sqred) instead of the PE:
# during the DMA ramp the PE is data-starved, and trimming its per-group
# work to the ramp arrival rate avoids the idle gaps that re-throttle the
# HAM clock gate (DVE is idle during the ramp)
RAMP_SQRED = 3
# endgame trigger points (t0+ntile values) and column chunks
EG_PLAN = [(32, 0, 32), (64, 32, 64), (96, 64, 96), (120, 96, 120)]
EG_FINAL = (120, 128)

F32 = mybir.dt.float32
BF16 = mybir.dt.bfloat16
AF = mybir.ActivationFunctionType
ALU = mybir.AluOpType

_cached_nc = {}


def _emit(ctx: ExitStack, tc, x_d, c_d, st_d, out_d, repeat: int = 1,
          hw_loop: int = 1):
    nc = tc.nc

    const = ctx.enter_context(tc.tile_pool(name="const", bufs=1))
    xbpool = ctx.enter_context(tc.tile_pool(name="xb", bufs=8))
    sqpool = ctx.enter_context(tc.tile_pool(name="sq", bufs=4))
    xtps = ctx.enter_context(tc.tile_pool(name="xtps", bufs=3, space="PSUM"))
    xtsb = ctx.enter_context(tc.tile_pool(name="xtsb", bufs=4))
    # G lives in 1-bank [P, 4, C] tiles (per-chunk min-reduce): four of
    # them rotate, giving the one-group-behind matmul pipeline ~3 chunks
    # of bank-reuse slack so the PE never waits on a min-reduce.
    gps = ctx.enter_context(tc.tile_pool(name="gps", bufs=4, space="PSUM"))
    scr_ps = ctx.enter_context(tc.tile_pool(name="scrps", bufs=1, space="PSUM"))
    endp = ctx.enter_context(tc.tile_pool(name="endp", bufs=1))

    # ---- one-time prep -------------------------------------------------
    # HAM warmup: dense matmuls on scratch data so the PE clock is at
    # 2.4 GHz by the time the real stream starts (~10us in)
    warm_rhs = const.tile([P, 4, P], BF16)
    nc.vector.memset(warm_rhs[:], 1.0)
    warm_ps = scr_ps.tile([P, 4, P], F32, tag="scratch")
    for _ in range(WARMUP_MMS):
        nc.tensor.matmul(warm_ps[:].rearrange("p t c -> p (t c)"),
                         lhsT=warm_rhs[:, 0, :],
                         rhs=warm_rhs[:].rearrange("p t c -> p (t c)"),
                         start=True, stop=True)

    # x loads are casting fp32->bf16 SWDGE DMAs (gpsimd): queue the first
    # groups immediately -- they need no prep and pace the whole kernel.
    pre_x8 = []
    for gd in range(3):
        lo, hi, tt = REGIONS[0]
        t0 = lo + gd * tt
        src0 = x_d[t0 * P:(t0 + tt) * P, :]
        src0 = src0.rearrange("(p t) d -> p t d", t=tt)
        x80 = xbpool.tile([P, DMA_GROUP, D], BF16, tag="xb")
        nc.gpsimd.dma_start(x80[:, :tt, :], src0)
        pre_x8.append(x80)

    ident_bf = const.tile([P, P], BF16)
    make_identity(nc, ident_bf[:])
    ident_f32 = const.tile([P, P], F32)
    make_identity(nc, ident_f32[:])

    # warm the ACT Square table set while DMAs are in flight (the
    # ACT_TABLE_LOAD costs ~2.7us and would otherwise sit in the prep
    # critical path at first use)
    warm = const.tile([1, 1], F32)
    nc.scalar.activation(warm[:], ident_f32[0:1, 0:1], AF.Square)

    c_sb = const.tile([C, D], F32)
    nc.sync.dma_start(c_sb[:], c_d[:])

    # c2 = rowsum(c^2) as a [128, 1] fp32 column
    c_sq = const.tile([C, D], F32)
    c2col = const.tile([C, 1], F32)
    nc.scalar.activation(c_sq[:], c_sb[:], AF.Square, accum_out=c2col[:])

    # (-2c) in bf16, then its transpose cT [d-chunk partitions, k, centers]
    cm2 = const.tile([C, D], BF16)
    nc.vector.tensor_scalar_mul(cm2[:], c_sb[:], -2.0)
    ct_ps = scr_ps.tile([P, 2, C], BF16, tag="scratch")
    for k in range(2):
        nc.tensor.transpose(ct_ps[:, k, :], cm2[:, k * P:(k + 1) * P], ident_bf[:])
    cT = const.tile([P, 2, C], BF16)
    nc.vector.tensor_copy(cT[:], ct_ps[:])

    # c2 as a single bf16 K-row for a K=1 ones-matmul (the old fp32-exact
    # hi+lo split needed a gpsimd casting DMA whose queue position stalled
    # the x-load triggers behind the whole prep chain; bf16 c2 costs ~1e-3
    # of the 2e-2 error budget)
    c2t_ps = scr_ps.tile([1, C], F32, tag="scratch")
    nc.tensor.transpose(c2t_ps[:], c2col[:], ident_f32[:])
    c2row_f = const.tile([1, C], F32)
    nc.vector.tensor_copy(c2row_f[:], c2t_ps[:])

    ones2 = const.tile([1, C], BF16)
    nc.vector.memset(ones2[:], 1.0)
    ones_col = const.tile([P, 1], F32)
    nc.vector.memset(ones_col[:], 1.0)

    # c2 row replicated PSUM_GROUP times for the single N=512 c2 matmul
    c2rows4 = const.tile([1, PSUM_GROUP, C], BF16)
    nc.vector.tensor_copy(c2rows4[:, 0, :], c2row_f[:])
    nc.vector.tensor_copy(c2rows4[:, 1, :], c2rows4[:, 0, :])
    nc.vector.tensor_copy(c2rows4[:, 2:4, :], c2rows4[:, 0:2, :])

    # all-ones [d, c] rhs for the PE-side x2 rank-1 accumulation
    ones_dc = const.tile([P, C], BF16)
    nc.vector.memset(ones_dc[:], 1.0)

    # semi_target: the HOST pre-permutes st into the x row mapping
    # (st_pre[p*NT + col] = st[row(p, col)], see make_in_maps), so one
    # contiguous 512B-per-partition DMA loads it.  The old direct load
    # of the scattered layout was 2048 32-byte descriptors (~14us) and
    # stalled the DVE queue behind the endgame's st-dependent ops.
    st_sb = const.tile([P, NT], F32)
    nc.sync.dma_start(st_sb[:], st_d[:].rearrange("(p j) -> p j", p=P))

    # per-b-tile accumulators: column j <-> b-tile j, partition p <-> row in tile
    mw = const.tile([P, NT], F32)
    x2w = const.tile([P, NT], F32)
    nc.vector.memset(x2w[:], 0.0)
    n_eg = len(EG_PLAN) + 1
    lsum2 = const.tile([P, n_eg], F32)

    # x2 rides in G via the PE for every group, so dist = max(mw, 0), and
    # loss = dist + min(st,0)*(dist - r) + max(st,0)*EPS  (r = 1/(dist+EPS))
    #      = dist*(1 + mneg) - mneg*r + epsq
    # with the st-only terms precomputed once here (mneg, m1 = 1+mneg) and
    # the epsq part folded into a single column sum (seg).
    m1 = const.tile([P, NT], F32)
    nc.vector.tensor_scalar(m1[:], st_sb[:], 0.0, 1.0, op0=ALU.min,
                            op1=ALU.add)
    mneg = const.tile([P, NT], F32)
    nc.vector.tensor_scalar_min(mneg[:], st_sb[:], 0.0)
    epsq = endp.tile([P, NT], F32, tag="epsq")
    nc.vector.tensor_scalar(epsq[:], st_sb[:], 0.0, EPS, op0=ALU.max,
                            op1=ALU.mult)
    seg = const.tile([P, 1], F32)
    nc.vector.tensor_reduce(seg[:], epsq[:], axis=mybir.AxisListType.X,
                            op=ALU.add)

    # ---- endgame (runs in chunks; all but the last overlap the main loop)
    def endgame_chunk(h, lo, hi):
        cols = slice(lo, hi)
        W = hi - lo
        dist = endp.tile([P, W], F32, tag=f"dist{h}")
        if lo < RAMP_SQRED * 4:
            # ramp groups keep x2 out of G (sqred path)
            nc.vector.tensor_tensor(dist[:], x2w[:, cols], mw[:, cols],
                                    op=ALU.add)
            nc.vector.tensor_scalar_max(dist[:], dist[:], 0.0)
        else:
            nc.vector.tensor_scalar_max(dist[:], mw[:, cols], 0.0)
        dp = endp.tile([P, W], F32, tag=f"dp{h}")
        nc.vector.tensor_scalar_add(dp[:], dist[:], EPS)
        r = endp.tile([P, W], F32, tag=f"r{h}")
        nc.vector.reciprocal(r[:], dp[:])
        a = endp.tile([P, W], F32, tag=f"a{h}")
        nc.vector.tensor_tensor(a[:], dist[:], m1[:, cols], op=ALU.mult)
        b = endp.tile([P, W], F32, tag=f"b{h}")
        nc.vector.tensor_tensor(b[:], r[:], mneg[:, cols], op=ALU.mult)
        losses = endp.tile([P, W], F32, tag=f"lo{h}")
        nc.vector.tensor_tensor(losses[:], a[:], b[:], op=ALU.subtract)
        nc.vector.tensor_reduce(lsum2[:, h:h + 1], losses[:],
                                axis=mybir.AxisListType.X, op=ALU.add)

    # ---- main loop -----------------------------------------------------
    if repeat == 1 and hw_loop == 1:
        plan = [(t, tt) for lo, hi, tt in REGIONS for t in range(lo, hi, tt)]
    else:
        plan = [(t, 8) for t in range(0, NT, 8)]

    def front_phase(pi, t0, ntile):
        """DMA + transposes + copies + squares for one group."""
        src = x_d[t0 * P:(t0 + ntile) * P, :]
        # row (p, t) of this group = batch t0*128 + p*ntile + t: each
        # partition reads one contiguous run per DMA
        src = src.rearrange("(p t) d -> p t d", t=ntile)
        if repeat == 1 and hw_loop == 1 and pi < len(pre_x8):
            x8 = pre_x8[pi]
        else:
            x8 = xbpool.tile([P, DMA_GROUP, D], BF16, tag="xb")
            nc.gpsimd.dma_start(x8[:, :ntile, :], src)

        cols = slice(t0, t0 + ntile)
        x2_on_pe = pi >= RAMP_SQRED
        chunks = [(s, min(s + PSUM_GROUP, ntile)) for s in
                  range(0, ntile, PSUM_GROUP)]

        if not x2_on_pe:
            # sqred path on DVE: TT square (bf16 2x), half-fold, reduce
            sq = sqpool.tile([P, DMA_GROUP, D], BF16, tag="sq")
            nc.vector.tensor_tensor(sq[:, :ntile, :], x8[:, :ntile, :],
                                    x8[:, :ntile, :], op=ALU.mult)
            s1 = sqpool.tile([P, DMA_GROUP, P], BF16, tag="s1")
            nc.vector.tensor_tensor(
                s1[:, :ntile, :], sq[:, :ntile, 0:P], sq[:, :ntile, P:D],
                op=ALU.add,
            )
            nc.vector.tensor_reduce(
                x2w[:, cols], s1[:, :ntile, :], axis=mybir.AxisListType.X,
                op=ALU.add,
            )

        # all transposes of the group (PE streams them densely)
        xt_pss = []
        for s, e in chunks:
            w = e - s
            xt_ps = xtps.tile([P, PSUM_GROUP, 2, P], BF16)
            for i in range(w):
                for k in range(2):
                    nc.tensor.transpose(
                        xt_ps[:, i, k, :], x8[:, s + i, k * P:(k + 1) * P],
                        ident_bf[:],
                    )
            xt_pss.append(xt_ps)
        # PSUM->SBUF pair copies (ACT)
        xt_ts = []
        for ci, (s, e) in enumerate(chunks):
            w = e - s
            xt_t = xtsb.tile([P, PSUM_GROUP, 2, P], BF16)
            nc.scalar.copy(xt_t[:, :w].bitcast(F32),
                           xt_pss[ci][:, :w].bitcast(F32))
            xt_ts.append(xt_t)
        return [t0, ntile, cols, x2_on_pe, chunks, xt_ts, None]

    def sqt_phase(state):
        """squares of the transposed tiles (DVE, bf16 2x) -- emitted
        AFTER the previous group's back_phase so the min-reduces sit in
        front of them in the DVE FIFO; otherwise the next group's
        x2-matmuls wait ~320ns on a queue-delayed sqt."""
        t0, ntile, cols, x2_on_pe, chunks, xt_ts, _ = state
        sqts = []
        for ci, (s, e) in enumerate(chunks):
            if not x2_on_pe:
                sqts.append(None)
                continue
            w = e - s
            sqt = sqpool.tile([P, PSUM_GROUP, 2, P], BF16, tag="sqt")
            nc.vector.tensor_tensor(sqt[:, :w], xt_ts[ci][:, :w],
                                    xt_ts[ci][:, :w], op=ALU.mult)
            sqts.append(sqt)
        state[6] = sqts

    def back_phase(state):
        """G matmuls + per-chunk min-reduce + endgame triggers, emitted
        one group behind front_phase: the PE streams this group's
        matmuls while the NEXT group's copies are in flight, removing
        the ~320ns copy-latency stall per bank."""
        t0, ntile, cols, x2_on_pe, chunks, xt_ts, sqts = state
        # both chunks' c2 matmuls first: same stationary operand (ones2),
        # so the second needs no weight reload, and the G-bank handoff
        # waits overlap
        g_chs = []
        for ci, (s, e) in enumerate(chunks):
            w = e - s
            g_ch = gps.tile([P, PSUM_GROUP, C], F32)
            nc.tensor.matmul(
                g_ch[:, :w].rearrange("p t c -> p (t c)"),
                lhsT=ones2[:],
                rhs=c2rows4[:, :w].rearrange("p t c -> p (t c)"),
                start=True, stop=False,
            )
            g_chs.append(g_ch)
        for ci, (s, e) in enumerate(chunks):
            w = e - s
            g_ch = g_chs[ci]
            for i in range(w):
                last_tile = i == w - 1
                nc.tensor.matmul(
                    g_ch[:, i, :], lhsT=xt_ts[ci][:, i, 0, :],
                    rhs=cT[:, 0, :], start=False, stop=False,
                )
                nc.tensor.matmul(
                    g_ch[:, i, :], lhsT=xt_ts[ci][:, i, 1, :],
                    rhs=cT[:, 1, :], start=False,
                    stop=(last_tile and not x2_on_pe),
                )
            if x2_on_pe:
                for i in range(w):
                    nc.tensor.matmul(
                        g_ch[:, i, :], lhsT=sqts[ci][:, i, 0, :],
                        rhs=ones_dc[:], start=False, stop=False,
                    )
                    nc.tensor.matmul(
                        g_ch[:, i, :], lhsT=sqts[ci][:, i, 1, :],
                        rhs=ones_dc[:], start=False, stop=(i == w - 1),
                    )
            nc.vector.tensor_reduce(
                mw[:, t0 + s:t0 + e], g_ch[:, :w, :],
                axis=mybir.AxisListType.X, op=ALU.min,
            )

        if repeat == 1 and hw_loop == 1:
            for h, (trig, lo, hi) in enumerate(EG_PLAN):
                if t0 + ntile == trig:
                    endgame_chunk(h, lo, hi)

    with tc.For_i(0, hw_loop, 1) if hw_loop > 1 else nullcontext():
     for _rep in range(repeat):
      prev_state = None
      for pi, (t0, ntile) in enumerate(plan):
        state = front_phase(pi, t0, ntile)
        if prev_state is not None:
            back_phase(prev_state)
        if repeat == 1 and hw_loop == 1 and RAMP_SQRED <= pi < RAMP_SQRED + 6:
            # the ramp->steady transition leaves the PE idle for ~2-3us
            # (pipeline startup bubble + DMA still ramping), long enough
            # for the HAM clock gate to re-throttle right before the
            # dense phase; these fillers run in that idle pocket
            for _ in range(3):
                nc.tensor.matmul(warm_ps[:].rearrange("p t c -> p (t c)"),
                                 lhsT=warm_rhs[:, 0, :],
                                 rhs=warm_rhs[:].rearrange("p t c -> p (t c)"),
                                 start=True, stop=True)
        if pi < RAMP_SQRED:
            # no pipelining during the DMA ramp: the PE is data-starved
            # there, and delaying the matmul phase just adds idle gaps
            # that re-throttle the HAM clock gate (ramp groups are
            # sqred, so they need no sqt)
            back_phase(state)
            prev_state = None
        else:
            sqt_phase(state)
            prev_state = state
      back_phase(prev_state)

    endgame_chunk(len(EG_PLAN), *EG_FINAL)
    lacc = lsum2[:, 0:1]
    lsum_t = None
    for h in range(1, n_eg):
        nxt = endp.tile([P, 1], F32, tag=f"ls{h}")
        nc.vector.tensor_tensor(nxt[:], lacc, lsum2[:, h:h + 1], op=ALU.add)
        lacc = nxt[:]
        lsum_t = nxt
    # single-descriptor 4-byte out DMA: a [128,1] out would be 128 tiny
    # descriptors whose completion receipt stalls the end barrier ~7us
    total_ps = scr_ps.tile([1, 1], F32, tag="scratch")
    nc.tensor.matmul(total_ps[:], lhsT=ones_col[:], rhs=lsum_t[:])
    total_sb = endp.tile([1, 1], F32)
    nc.vector.tensor_copy(total_sb[:], total_ps[:])
    nc.sync.dma_start(out_d[:], total_sb[:])


def build_nc(repeat: int = 1, hw_loop: int = 1, internal_x: bool = False):
    key = (repeat, hw_loop, internal_x)
    if key in _cached_nc:
        return _cached_nc[key]
    nc = bacc.Bacc(
        "TRN2",
        target_bir_lowering=False,
        debug=False,
        enable_asserts=False,
        num_devices=N_CORES,
    )
    if internal_x:
        # timing-only builds: x is internal (uninitialized) DRAM so bench
        # calls don't upload 128 MiB; compute timing is data-independent
        x_d = nc.dram_tensor("x", [B_SH, D], F32).ap()
    else:
        x_d = nc.dram_tensor("x", [B_SH, D], F32, kind="ExternalInput").ap()
    c_d = nc.dram_tensor("c", [C, D], F32, kind="ExternalInput").ap()
    st_d = nc.dram_tensor("st", [B_SH], F32, kind="ExternalInput").ap()
    out_d = nc.dram_tensor("out", [1, 1], F32, kind="ExternalOutput").ap()

    with tile.TileContext(nc) as tc:
        with ExitStack() as ctx:
            _emit(ctx, tc, x_d, c_d, st_d, out_d, repeat=repeat, hw_loop=hw_loop)
    nc.compile()
    _cached_nc[key] = nc
    return nc


_ST_IDX = None


def _st_index():
    # row index feeding st_sb[p, col]: in an ntile-tile group at tile t0,
    # batch row t0*128 + p*ntile + t sits at column t0 + t
    global _ST_IDX
    if _ST_IDX is None:
        idx = np.empty((P, NT), dtype=np.int64)
        p = np.arange(P)[:, None]
        for lo, hi, tt in REGIONS:
            for g0 in range(lo, hi, tt):
                t = np.arange(tt)[None, :]
                idx[:, g0:g0 + tt] = g0 * P + p * tt + t
        _ST_IDX = idx.ravel()
    return _ST_IDX


def make_in_maps(x, c, stf):
    idx = _st_index()
    return [
        {
            "x": np.ascontiguousarray(x[i * B_SH:(i + 1) * B_SH]),
            "c": c,
            "st": np.ascontiguousarray(stf[i * B_SH:(i + 1) * B_SH][idx]),
        }
        for i in range(N_CORES)
    ]


def kernel(**inputs) -> np.ndarray:
    x = np.ascontiguousarray(np.asarray(inputs["input"], dtype=np.float32))
    c = np.ascontiguousarray(np.asarray(inputs["c"], dtype=np.float32))
    stf = np.asarray(inputs["semi_target"]).astype(np.float32)

    nc = build_nc()
    res = run_bass_kernel_spmd(nc, make_in_maps(x, c, stf), list(range(N_CORES)))
    total = sum(float(r["out"][0, 0]) for r in res.results)
    return np.asarray(np.float32(total / B))
